# revision 1
# baseline (speedup 1.0000x reference)
"""Causal single-head attention (B=4, S=4096, d=1024) on 8 Trainium2 NeuronCores.

Sharding: 8 cores = 4 batches x 2 sequence-groups.  Per batch, the 8 causal
q-blocks of 512 rows (k-tile coverage 4,8,...,32) are paired (4,8), (12,16),
(20,24), (28,32) so each core gets 72 real k-tile visits, padded up to one
uniform static program with slot coverages (8,16,24,32) = 80 visits.  Causal
masking and the padding are handled by a data-driven mask
    A = exp(s/sqrt(d)) * (I - J <= delta)
so all 8 cores run a single SPMD program; only the input data differs per core.

Optionally (rg=...) the two cores of a batch each project k/v for HALF the
sequence and exchange halves with an AllGather, halving projection flops.

Math (per core):
  qT = Wq^T xq^T, kT = Wk^T x^T (both [d, s], d on partitions), v = x Wv.
  Per slot (512 q rows), per k-tile (128 rows):
    sT[k, q]   = sum_e kT[e,k] qT[e,q]          (PE, fp32r, N=512)
    A[k, q]    = exp(sT/32) * mask              (ACT exp PSUM->SBUF, DVE mask)
    out[q, d] += A[:,qm]^T v[k, d]              (PE, accumulated in PSUM)
    den[q]    += A[:,qm]^T ones                 (PE, N=2 - fp32r needs N>=2)
  out /= den  (DVE reciprocal + ACT Copy with per-partition scale)
No running max is needed: scores are ~N(0,1) after the 1/32 scale, and exp
without max-subtraction is exact in fp32 here.

The attention inner loop is software-pipelined: every DMA is issued at least
one step ahead, and visit i's attn@v/denominator matmuls are emitted after
visit i+1's score matmuls so the PE never waits on the ACT exp.

Toolchain notes: fp32r matmul inputs must be produced as float32r (BIR
verifier); self-loading fp32r matmuls allow one sync wait, so tiny PE
"pre-touch" matmuls observe each fresh DMA tick; bacc.Bacc legalizes any
remaining multi-wait instructions via event semaphores.
"""

import contextlib
import math

import numpy as np

import concourse.bass as bass  # noqa: F401
import concourse.mybir as mybir
import concourse.tile as tile
from concourse import bacc
from concourse.bass_utils import run_bass_kernel_spmd

F32 = mybir.dt.float32
F32R = mybir.dt.float32r
AF = mybir.ActivationFunctionType
ALU = mybir.AluOpType

CFG_FULL = dict(S=4096, D=1024, QBLK=512, COV=(8, 16, 24, 32))
Q0_FULL = {0: (0, 1536, 2048, 3584), 1: (512, 1024, 2560, 3072)}
RG_FULL = [[0, 1], [2, 3], [4, 5], [6, 7]]
B_FULL = 4
USE_RG = False  # AllGather kv-dedup; off by default (collective cost)


def build_nc(S, D, QBLK, COV, reps=1, rg=None):
    """Build the single-core Bass program (identical across all cores)."""
    DC = D // 128
    M = QBLK // 128
    nslots = len(COV)
    QROWS = nslots * QBLK
    DHALF = min(512, D)
    NH = D // DHALF
    NR = 2 if rg else 1
    SH = S // NR                 # per-core projected kv rows
    SBLK = min(512, SH)
    NSB = SH // SBLK
    maxcov = max(COV)
    assert maxcov == S // 128
    G = 4
    while any(c % G for c in COV):
        G //= 2
    assert (SH // 128) % G == 0
    scale = 1.0 / math.sqrt(D)

    ndev = (max(max(g) for g in rg) + 1) if rg else None
    nc = bacc.Bacc("TRN2", target_bir_lowering=False, num_devices=ndev)
    xT_d = nc.dram_tensor("xT", [D, SH], F32, kind="ExternalInput")
    xTq_d = nc.dram_tensor("xTq", [D, QROWS], F32, kind="ExternalInput")
    wq_d = nc.dram_tensor("Wq", [D, D], F32, kind="ExternalInput")
    wk_d = nc.dram_tensor("Wk", [D, D], F32, kind="ExternalInput")
    wv_d = nc.dram_tensor("Wv", [D, D], F32, kind="ExternalInput")
    ij_d = nc.dram_tensor("IJ", [128, QBLK], F32, kind="ExternalInput")
    dl_d = nc.dram_tensor("delta", [128, nslots * maxcov], F32,
                          kind="ExternalInput")
    ones_d = nc.dram_tensor("ones", [128, 2], F32, kind="ExternalInput")
    out_d = nc.dram_tensor("out", [QROWS, D], F32, kind="ExternalOutput")

    def dpart(ap):
        return ap.rearrange("(c p) n -> p c n", p=128)

    with tile.TileContext(nc) as tc:
        with tc.tile_pool(name="dram", bufs=1, space="DRAM") as dram, \
             tc.tile_pool(name="dummy", bufs=1, space="PSUM") as dummypool:
            kTh_i = dram.tile([DC, 128, SH], F32, name="kTh_i")
            vh_i = dram.tile([SH, D], F32, name="vh_i")
            if rg:
                kT_i = dram.tile([NR, DC, 128, SH], F32, name="kT_i")
                v_i = dram.tile([NR, SH, D], F32, name="v_i")
            else:
                kT_i, v_i = kTh_i, vh_i
            qT_i = dram.tile([DC, 128, QROWS], F32, name="qT_i")
            dummy_ps = dummypool.tile([128, 2], F32, name="dummy_ps",
                                      tag="dummy")

            def touch(cols2):
                # Tiny matmul reading two columns of a freshly written SBUF
                # tile: walrus allows ONE sync wait on self-loading fp32r
                # matmuls, so the PE observes DMA ticks via these.
                nc.tensor.matmul(dummy_ps[0:1, 0:2], cols2[:, 0:1], cols2,
                                 start=True, stop=True)

            def kt_src(g):
                lo = g * G * 128
                if rg:
                    half, loc = lo // SH, lo % SH
                    apv = kT_i[half, :, :, loc:loc + G * 128]
                else:
                    apv = kT_i[:, :, lo:lo + G * 128]
                return apv.rearrange("c p y -> p c y").bitcast(F32R)

            def v_src(i, d0, dn):
                lo = i * 128
                if rg:
                    half, loc = lo // SH, lo % SH
                    apv = v_i[half, loc:loc + 256, d0:d0 + dn]
                else:
                    apv = v_i[lo:lo + 256, d0:d0 + dn]
                return apv.rearrange("(t p) d -> p t d", p=128).bitcast(F32R)

            _loop = (tc.For_i(0, reps, 1) if reps > 1
                     else contextlib.nullcontext())
            with _loop:
                # ---------------- Phase 1: projections ----------------
                with (
                    tc.tile_pool(name="w", bufs=1) as wpool,
                    tc.tile_pool(name="xt", bufs=3) as xtpool,
                    tc.tile_pool(name="kst", bufs=6) as kspool,
                    tc.tile_pool(name="vst", bufs=4) as vspool,
                    tc.tile_pool(name="ppsum", bufs=7, space="PSUM") as ppsum,
                ):
                    w_sb = {}
                    for name, wd in (("q", wq_d), ("k", wk_d), ("v", wv_d)):
                        w_sb[name] = wpool.tile([128, DC, D], F32R,
                                                name=f"w{name}",
                                                tag=f"w{name}")
                        nc.sync.dma_start(out=w_sb[name],
                                          in_=dpart(wd[:, :]).bitcast(F32R))
                        touch(w_sb[name][:, 0, 0:2])

                    def pcopy(dst, src):
                        # PSUM->SBUF on ACT so the store (also issued from
                        # ACT's HWDGE queue) needs no cross-engine wait.
                        nc.scalar.copy(out=dst, in_=src)

                    # one prefetched stream of xT blocks.  Interleave kv
                    # and q jobs so attention slot s's inputs (qT slot s +
                    # kv blocks covering COV[s] k-tiles) are stored as early
                    # as possible: slot 0 unblocks ~25% into projection.
                    kvjobs = [("kv", sb) for sb in range(NSB)]
                    qjobs = [("q", s) for s in range(nslots)]
                    jobs = []
                    per = max(1, NSB // nslots)
                    for s in range(nslots):
                        jobs += kvjobs[s * per:(s + 1) * per]
                        jobs.append(qjobs[s])
                    jobs += kvjobs[nslots * per:]

                    def xt_load(job):
                        kind, idx = job
                        blk = SBLK if kind == "kv" else QBLK
                        src = xT_d if kind == "kv" else xTq_d
                        xt = xtpool.tile([128, DC, blk], F32R, name="xt",
                                         tag="xt")
                        nc.sync.dma_start(
                            out=xt,
                            in_=dpart(src[:, idx * blk:(idx + 1) * blk])
                            .bitcast(F32R))
                        return xt

                    xts = {0: xt_load(jobs[0])}
                    for jidx, job in enumerate(jobs):
                        if jidx + 1 < len(jobs):
                            xts[jidx + 1] = xt_load(jobs[jidx + 1])
                        xt = xts.pop(jidx)
                        touch(xt[:, 0, 0:2])
                        kind, idx = job
                        if kind == "kv":
                            for co in range(DC):
                                ps = ppsum.tile([128, SBLK], F32, name="pp",
                                                tag="pp")
                                for ci in range(DC):
                                    nc.tensor.matmul(
                                        ps,
                                        w_sb["k"][:, ci,
                                                  co * 128:(co + 1) * 128],
                                        xt[:, ci, :],
                                        start=(ci == 0), stop=(ci == DC - 1))
                                ks = kspool.tile([128, SBLK], F32, name="ks",
                                                 tag="ks")
                                pcopy(ks, ps)
                                nc.scalar.dma_start(
                                    out=kTh_i[co, :,
                                              idx * SBLK:(idx + 1) * SBLK],
                                    in_=ks)
                            for m in range(SBLK // 128):
                                vs = vspool.tile([128, D], F32, name="vs",
                                                 tag="vs")
                                for h in range(NH):
                                    ps = ppsum.tile([128, DHALF], F32,
                                                    name="pp", tag="pp")
                                    for ci in range(DC):
                                        nc.tensor.matmul(
                                            ps,
                                            xt[:, ci, m * 128:(m + 1) * 128],
                                            w_sb["v"][:, ci, h * DHALF:
                                                      (h + 1) * DHALF],
                                            start=(ci == 0),
                                            stop=(ci == DC - 1))
                                    pcopy(vs[:, h * DHALF:(h + 1) * DHALF],
                                          ps)
                                nc.scalar.dma_start(
                                    out=vh_i[idx * SBLK + m * 128:
                                             idx * SBLK + (m + 1) * 128, :],
                                    in_=vs)
                            if rg and idx == NSB - 1:
                                # kv halves done: exchange within the pair
                                nc.gpsimd.collective_compute(
                                    "AllGather", ALU.bypass,
                                    replica_groups=rg,
                                    ins=[kTh_i[:, :, :]],
                                    outs=[kT_i[:, :, :, :]])
                                nc.gpsimd.collective_compute(
                                    "AllGather", ALU.bypass,
                                    replica_groups=rg,
                                    ins=[vh_i[:, :]], outs=[v_i[:, :, :]])
                        else:
                            for co in range(DC):
                                ps = ppsum.tile([128, QBLK], F32, name="pp",
                                                tag="pp")
                                for ci in range(DC):
                                    nc.tensor.matmul(
                                        ps,
                                        w_sb["q"][:, ci,
                                                  co * 128:(co + 1) * 128],
                                        xt[:, ci, :],
                                        start=(ci == 0), stop=(ci == DC - 1))
                                ks = kspool.tile([128, QBLK], F32, name="ks",
                                                 tag="ks")
                                pcopy(ks, ps)
                                nc.scalar.dma_start(
                                    out=qT_i[co, :,
                                             idx * QBLK:(idx + 1) * QBLK],
                                    in_=ks)

                # ---------------- Phase 2: attention ----------------
                with (
                    tc.tile_pool(name="qt", bufs=2) as qtpool,
                    tc.tile_pool(name="kt", bufs=2) as ktpool,
                    tc.tile_pool(name="at", bufs=maxcov) as apool,
                    tc.tile_pool(name="vt", bufs=4) as vtpool,
                    tc.tile_pool(name="ot", bufs=4) as otpool,
                    tc.tile_pool(name="cm", bufs=2) as cmpool,
                    tc.tile_pool(name="sm", bufs=1) as smpool,
                    tc.tile_pool(name="rc", bufs=2) as rcpool,
                    tc.tile_pool(name="spsum", bufs=2, space="PSUM") as spsum,
                    tc.tile_pool(name="opsum", bufs=M, space="PSUM") as opsum,
                    tc.tile_pool(name="dpsum", bufs=1, space="PSUM") as dpsum,
                ):
                    ij_sb = smpool.tile([128, QBLK], F32, name="ij", tag="ij")
                    nc.sync.dma_start(out=ij_sb, in_=ij_d[:, :])
                    dl_sb = smpool.tile([128, nslots * maxcov], F32,
                                        name="dl", tag="dl")
                    nc.sync.dma_start(out=dl_sb, in_=dl_d[:, :])
                    ones_sb = smpool.tile([128, 2], F32R, name="ones",
                                          tag="ones")
                    nc.sync.dma_start(out=ones_sb,
                                      in_=ones_d[:, :].bitcast(F32R))
                    touch(ones_sb)

                    def qt_load(s):
                        qt = qtpool.tile([128, DC, QBLK], F32R, name="qt",
                                         tag="qt")
                        nc.sync.dma_start(
                            out=qt,
                            in_=qT_i[:, :, s * QBLK:(s + 1) * QBLK]
                            .rearrange("c p y -> p c y").bitcast(F32R))
                        return qt

                    def kt_load(g):
                        kt = ktpool.tile([128, DC, G * 128], F32R, name="kt",
                                         tag="kt")
                        nc.sync.dma_start(out=kt, in_=kt_src(g))
                        return kt

                    def vt_load(i, h):
                        vt = vtpool.tile([128, 2, DHALF], F32R, name="vt",
                                         tag="vt")
                        nc.sync.dma_start(out=vt,
                                          in_=v_src(i, h * DHALF, DHALF))
                        return vt

                    # slot-0 prologue loads
                    qt_next = qt_load(0)
                    kt_next = kt_load(0)
                    for s in range(nslots):
                        cov = COV[s]
                        qt, qt_next = qt_next, None
                        touch(qt[:, 0, 0:2])
                        po = [opsum.tile([128, DHALF], F32, name="po",
                                         tag="po") for _ in range(M)]
                        pd = dpsum.tile([128, 2 * M], F32, name="pd",
                                        tag="pd")

                        def consume(i, at, vts, hh, dst, fresh_touch):
                            # attn@v + denominator matmuls for visit i
                            vt = vts[i // 2]
                            if fresh_touch:
                                touch(vt[:, 0, 0:2])
                            for m in range(M):
                                nc.tensor.matmul(
                                    dst[m],
                                    at[:, m * 128:(m + 1) * 128],
                                    vt[:, i % 2, :],
                                    start=(i == 0), stop=(i == cov - 1))
                            if hh == 0:
                                for m in range(M):
                                    nc.tensor.matmul(
                                        pd[:, 2 * m:2 * m + 2],
                                        at[:, m * 128:(m + 1) * 128],
                                        ones_sb[:, :],
                                        start=(i == 0 and m == 0),
                                        stop=(i == cov - 1 and m == M - 1))

                        # ---- sweep 1: scores + exp + attn@v(d-half 0) ----
                        a_tiles = []
                        vts = {0: vt_load(0, 0)}
                        kt = kt_next
                        prev = None
                        for i in range(cov):
                            g, j = divmod(i, G)
                            if j == 0:
                                if g > 0:
                                    kt = kt_next
                                touch(kt[:, 0, 0:2])
                            if j == 1 and g + 1 < cov // G:
                                kt_next = kt_load(g + 1)
                            if i % 2 == 0 and i + 2 < cov:
                                vts[(i + 2) // 2] = vt_load(i + 2, 0)
                            ps = spsum.tile([128, QBLK], F32, name="ps",
                                            tag="ps")
                            for ci in range(DC):
                                nc.tensor.matmul(
                                    ps,
                                    kt[:, ci, j * 128:(j + 1) * 128],
                                    qt[:, ci, :],
                                    start=(ci == 0), stop=(ci == DC - 1))
                            at = apool.tile([128, QBLK], F32R, name="at",
                                            tag="at")
                            nc.scalar.activation(
                                out=at, in_=ps, func=AF.Exp, scale=scale)
                            if i >= cov - 2 * M:
                                cm = cmpool.tile([128, QBLK], F32R,
                                                 name="cm", tag="cm")
                                nc.vector.tensor_scalar(
                                    out=cm, in0=ij_sb,
                                    scalar1=dl_sb[:, s * maxcov + i:
                                                  s * maxcov + i + 1],
                                    scalar2=None, op0=ALU.is_le)
                                nc.vector.tensor_mul(out=at, in0=at, in1=cm)
                            a_tiles.append(at)
                            if prev is not None:
                                consume(prev, a_tiles[prev], vts, 0, po,
                                        prev % 2 == 0)
                            prev = i
                        consume(prev, a_tiles[prev], vts, 0, po,
                                prev % 2 == 0)

                        rc = rcpool.tile([128, 2 * M], F32, name="rc",
                                         tag="rc")
                        nc.vector.reciprocal(out=rc, in_=pd)

                        # prefetch next slot's qt/kt while sweep 2 runs
                        if s + 1 < nslots:
                            qt_next = qt_load(s + 1)
                            kt_next = kt_load(0)

                        def norm_store(dst_po, h):
                            for m in range(M):
                                ot = otpool.tile([128, DHALF], F32,
                                                 name="ot", tag="ot")
                                nc.scalar.activation(
                                    out=ot, in_=dst_po[m], func=AF.Copy,
                                    scale=rc[:, 2 * m:2 * m + 1])
                                nc.scalar.dma_start(
                                    out=out_d[s * QBLK + m * 128:
                                              s * QBLK + (m + 1) * 128,
                                              h * DHALF:(h + 1) * DHALF],
                                    in_=ot)

                        norm_store(po, 0)

                        # ---- sweep 2: attn@v for remaining d-halves ----
                        for h in range(1, NH):
                            po2 = [opsum.tile([128, DHALF], F32, name="po",
                                              tag="po") for _ in range(M)]
                            vts = {0: vt_load(0, h)}
                            if cov > 2:
                                vts[1] = vt_load(2, h)
                            for i in range(cov):
                                if i % 2 == 0 and i + 4 < cov:
                                    vts[(i + 4) // 2] = vt_load(i + 4, h)
                                consume(i, a_tiles[i], vts, h, po2,
                                        i % 2 == 0)
                            norm_store(po2, h)
    # Bacc legalization: splits >1 sync waits into event semaphores and moves
    # excess matmul waits (walrus allows 1 wait per instruction).
    nc.compile()
    return nc


def host_core_inputs(x_b, Wq, Wk, Wv, q0s, S, D, QBLK, COV, half=None):
    """Input map for one core.  half=None: full-sequence kv projection;
    half=0/1: this core projects kv rows [half*S/2, (half+1)*S/2)."""
    nslots = len(COV)
    maxcov = max(COV)
    if half is None:
        xT = np.ascontiguousarray(x_b.T)
    else:
        SH = S // 2
        xT = np.ascontiguousarray(x_b[half * SH:(half + 1) * SH].T)
    xTq = np.ascontiguousarray(
        np.concatenate([x_b[q0:q0 + QBLK] for q0 in q0s], axis=0).T)
    ij = (np.arange(128, dtype=np.float32)[:, None]
          - np.arange(QBLK, dtype=np.float32)[None, :])
    ij = np.ascontiguousarray(np.broadcast_to(ij, (128, QBLK)))
    delta = np.empty((nslots, maxcov), dtype=np.float32)
    for s, q0 in enumerate(q0s):
        delta[s, :] = q0 - 128.0 * np.arange(maxcov, dtype=np.float32)
    delta = np.ascontiguousarray(
        np.broadcast_to(delta.reshape(1, -1), (128, nslots * maxcov)))
    return {
        "xT": xT, "xTq": xTq,
        "Wq": np.ascontiguousarray(Wq, np.float32),
        "Wk": np.ascontiguousarray(Wk, np.float32),
        "Wv": np.ascontiguousarray(Wv, np.float32),
        "IJ": ij, "delta": delta,
        "ones": np.ones((128, 2), np.float32),
    }


_NC_CACHE = {}


def _get_nc(key, cfg, **kw):
    if key not in _NC_CACHE:
        _NC_CACHE[key] = build_nc(**cfg, **kw)
    return _NC_CACHE[key]


def run_full(x, Wq, Wk, Wv, trace=False, trace_cores=None):
    """Run the 8-core kernel on full inputs; returns (out, BassKernelResults)."""
    cfg = CFG_FULL
    S, D, QBLK, COV = cfg["S"], cfg["D"], cfg["QBLK"], cfg["COV"]
    x = np.asarray(x, np.float32)
    Wq = np.asarray(Wq, np.float32)
    Wk = np.asarray(Wk, np.float32)
    Wv = np.asarray(Wv, np.float32)
    B = x.shape[0]
    assert (B, x.shape[1], x.shape[2]) == (B_FULL, S, D)

    rg = RG_FULL if USE_RG else None
    nc = _get_nc("full", cfg, rg=rg)
    in_maps = []
    for b in range(B):
        for h in range(2):
            in_maps.append(host_core_inputs(
                x[b], Wq, Wk, Wv, Q0_FULL[h], S, D, QBLK, COV,
                half=(h if rg else None)))
    res = run_bass_kernel_spmd(
        nc, in_maps, list(range(2 * B)), trace=trace,
        trace_cores=trace_cores)
    out = np.empty((B, S, D), np.float32)
    for b in range(B):
        for h in range(2):
            o = np.asarray(res.results[2 * b + h]["out"])
            for s, q0 in enumerate(Q0_FULL[h]):
                out[b, q0:q0 + QBLK] = o[s * QBLK:(s + 1) * QBLK]
    return out, res


def kernel(x, Wq, Wk, Wv):
    out, _ = run_full(x, Wq, Wk, Wv)
    return out



# revision 2
# speedup vs baseline: 7.2605x; 7.2605x over previous
"""Causal single-head attention (B=4, S=4096, d=1024) on 8 Trainium2 NeuronCores.

Sharding: 8 cores = 4 batches x 2 sequence-groups.  Per batch, the causal
q-blocks are paired so each core gets a uniform static program with slot
coverages COV; causal masking and padding are handled by a data-driven mask
    A = exp(s/sqrt(d)) * (I - J <= delta)
so all 8 cores run a single SPMD program; only the input data differs per core.

v2 (this file): everything the PE touches is bf16.
  - Host converts x^T and Wq/Wk/Wv to bf16 (PE bf16 matmul = same 1 col/cycle
    as fp32r, but half the DMA/SBUF bytes; rel-err budget 2e-2 >> bf16's ~5e-3).
  - kT and qT live in SBUF for the whole kernel ([128, DC, S] + [128, DC, QROWS]
    bf16 = 96 KiB/partition): k/q projection's PSUM->SBUF ACT copies write them
    directly, so the attention phase has NO kt/qt DMA at all.
  - v goes to DRAM in bf16 and is streamed during attention (~20 MB/core vs
    ~88 MB/core f32 before) -- far below the ~360 GB/s/core HBM ceiling the
    old version was pinned at.

Math (per core):
  kT = Wk^T x^T ([d, s], d on partitions) -> SBUF bf16; v = x Wv -> DRAM bf16;
  qT = Wq^T xq^T -> SBUF bf16.
  Per slot (QBLK q rows), per k-tile (128 rows):
    sT[k, q]   = sum_e kT[e,k] qT[e,q]          (PE, bf16, fp32 PSUM)
    A[k, q]    = exp(sT/32) * mask              (ACT exp PSUM->SBUF bf16, DVE)
    out[q, d] += A[:,qm]^T v[k, d]              (PE, accumulated in PSUM)
    den[q]    += A[:,qm]^T ones                 (PE, N=2)
  out /= den  (DVE reciprocal + ACT Copy with per-partition scale)
No running max is needed: scores are ~N(0,1) after the 1/32 scale, and exp
without max-subtraction is safe (max score ~5.5 -> exp ~250, well in range).

The attention inner loop is software-pipelined: every DMA is issued at least
one step ahead, and visit i's attn@v/denominator matmuls are emitted after
visit i+1's score matmuls so the PE never waits on the ACT exp.

Toolchain notes: tiny PE "pre-touch" matmuls observe each fresh DMA tick so
real matmuls keep a single sync wait; bacc.Bacc legalizes any remaining
multi-wait instructions via event semaphores.
"""

import contextlib
import math

import numpy as np
from ml_dtypes import bfloat16

import concourse.bass as bass  # noqa: F401
import concourse.mybir as mybir
import concourse.tile as tile
from concourse import bacc
from concourse.bass_utils import run_bass_kernel_spmd

F32 = mybir.dt.float32
BF16 = mybir.dt.bfloat16
AF = mybir.ActivationFunctionType
ALU = mybir.AluOpType

CFG_FULL = dict(S=4096, D=1024, QBLK=512, COV=(8, 16, 24, 32))
Q0_FULL = {0: (0, 1536, 2048, 3584), 1: (512, 1024, 2560, 3072)}
RG_FULL = [[0, 1], [2, 3], [4, 5], [6, 7]]
B_FULL = 4
USE_RG = False  # collectives too slow for kv-dedup (2-rank AG ~34 GB/s)


def build_nc(S, D, QBLK, COV, reps=1, rg=None):
    """Build the single-core Bass program (identical across all cores)."""
    assert rg is None
    DC = D // 128
    M = QBLK // 128
    nslots = len(COV)
    QROWS = nslots * QBLK
    DHALF = min(512, D)
    NH = D // DHALF
    SBLK = min(512, S)
    NSB = S // SBLK
    maxcov = max(COV)
    assert maxcov == S // 128
    scale = 1.0 / math.sqrt(D)

    nc = bacc.Bacc("TRN2", target_bir_lowering=False)
    xT_d = nc.dram_tensor("xT", [D, S], BF16, kind="ExternalInput")
    xTq_d = nc.dram_tensor("xTq", [D, QROWS], BF16, kind="ExternalInput")
    wq_d = nc.dram_tensor("Wq", [D, D], BF16, kind="ExternalInput")
    wk_d = nc.dram_tensor("Wk", [D, D], BF16, kind="ExternalInput")
    wv_d = nc.dram_tensor("Wv", [D, D], BF16, kind="ExternalInput")
    ij_d = nc.dram_tensor("IJ", [128, QBLK], F32, kind="ExternalInput")
    dl_d = nc.dram_tensor("delta", [128, nslots * maxcov], F32,
                          kind="ExternalInput")
    ones_d = nc.dram_tensor("ones", [128, 2], BF16, kind="ExternalInput")
    out_d = nc.dram_tensor("out", [QROWS, D], F32, kind="ExternalOutput")

    def dpart(ap):
        return ap.rearrange("(c p) n -> p c n", p=128)

    with tile.TileContext(nc) as tc:
        with tc.tile_pool(name="persist", bufs=1) as pers, \
             tc.tile_pool(name="dram", bufs=1, space="DRAM") as dram, \
             tc.tile_pool(name="dummy", bufs=1, space="PSUM") as dummypool:
            kT_sb = pers.tile([128, DC, S], BF16, name="kT", tag="kT")
            qT_sb = pers.tile([128, DC, QROWS], BF16, name="qT", tag="qT")
            v_i = dram.tile([S, D], BF16, name="v_i")
            dummy_ps = dummypool.tile([128, 2], F32, name="dummy_ps",
                                      tag="dummy")

            def touch(cols2):
                # Tiny matmul reading two columns of a freshly written SBUF
                # tile: absorbs the DMA-completion wait so the real matmuls
                # keep a single sync wait each.
                nc.tensor.matmul(dummy_ps[0:1, 0:2], cols2[:, 0:1], cols2,
                                 start=True, stop=True)

            _loop = (tc.For_i(0, reps, 1) if reps > 1
                     else contextlib.nullcontext())
            with _loop:
                # ---------------- Phase 1: projections ----------------
                with (
                    tc.tile_pool(name="w", bufs=1) as wpool,
                    tc.tile_pool(name="xt", bufs=3) as xtpool,
                    tc.tile_pool(name="vst", bufs=4) as vspool,
                    tc.tile_pool(name="ppsum", bufs=7, space="PSUM") as ppsum,
                ):
                    w_sb = {}
                    for name, wd in (("q", wq_d), ("k", wk_d), ("v", wv_d)):
                        w_sb[name] = wpool.tile([128, DC, D], BF16,
                                                name=f"w{name}",
                                                tag=f"w{name}")
                        nc.sync.dma_start(out=w_sb[name], in_=dpart(wd[:, :]))
                        touch(w_sb[name][:, 0, 0:2])

                    def pcopy(dst, src):
                        # PSUM->SBUF on ACT (casts f32 PSUM to bf16 dst).
                        nc.scalar.copy(out=dst, in_=src)

                    # one prefetched stream of xT blocks.  Interleave kv
                    # and q jobs so attention slot s's inputs are stored as
                    # early as possible.
                    kvjobs = [("kv", sb) for sb in range(NSB)]
                    qjobs = [("q", s) for s in range(nslots)]
                    jobs = []
                    per = max(1, NSB // nslots)
                    for s in range(nslots):
                        jobs += kvjobs[s * per:(s + 1) * per]
                        jobs.append(qjobs[s])
                    jobs += kvjobs[nslots * per:]

                    def xt_load(job):
                        kind, idx = job
                        blk = SBLK if kind == "kv" else QBLK
                        src = xT_d if kind == "kv" else xTq_d
                        xt = xtpool.tile([128, DC, blk], BF16, name="xt",
                                         tag="xt")
                        nc.sync.dma_start(
                            out=xt,
                            in_=dpart(src[:, idx * blk:(idx + 1) * blk]))
                        return xt

                    xts = {0: xt_load(jobs[0])}
                    for jidx, job in enumerate(jobs):
                        if jidx + 1 < len(jobs):
                            xts[jidx + 1] = xt_load(jobs[jidx + 1])
                        xt = xts.pop(jidx)
                        touch(xt[:, 0, 0:2])
                        kind, idx = job
                        if kind == "kv":
                            for co in range(DC):
                                ps = ppsum.tile([128, SBLK], F32, name="pp",
                                                tag="pp")
                                for ci in range(DC):
                                    nc.tensor.matmul(
                                        ps,
                                        w_sb["k"][:, ci,
                                                  co * 128:(co + 1) * 128],
                                        xt[:, ci, :],
                                        start=(ci == 0), stop=(ci == DC - 1))
                                pcopy(kT_sb[:, co,
                                            idx * SBLK:(idx + 1) * SBLK], ps)
                            for m in range(SBLK // 128):
                                vs = vspool.tile([128, D], BF16, name="vs",
                                                 tag="vs")
                                for h in range(NH):
                                    ps = ppsum.tile([128, DHALF], F32,
                                                    name="pp", tag="pp")
                                    for ci in range(DC):
                                        nc.tensor.matmul(
                                            ps,
                                            xt[:, ci, m * 128:(m + 1) * 128],
                                            w_sb["v"][:, ci, h * DHALF:
                                                      (h + 1) * DHALF],
                                            start=(ci == 0),
                                            stop=(ci == DC - 1))
                                    pcopy(vs[:, h * DHALF:(h + 1) * DHALF],
                                          ps)
                                nc.scalar.dma_start(
                                    out=v_i[idx * SBLK + m * 128:
                                            idx * SBLK + (m + 1) * 128, :],
                                    in_=vs)
                        else:
                            for co in range(DC):
                                ps = ppsum.tile([128, QBLK], F32, name="pp",
                                                tag="pp")
                                for ci in range(DC):
                                    nc.tensor.matmul(
                                        ps,
                                        w_sb["q"][:, ci,
                                                  co * 128:(co + 1) * 128],
                                        xt[:, ci, :],
                                        start=(ci == 0), stop=(ci == DC - 1))
                                pcopy(qT_sb[:, co,
                                            idx * QBLK:(idx + 1) * QBLK], ps)

                # ---------------- Phase 2: attention ----------------
                with (
                    tc.tile_pool(name="at", bufs=maxcov) as apool,
                    tc.tile_pool(name="vt", bufs=4) as vtpool,
                    tc.tile_pool(name="ot", bufs=4) as otpool,
                    tc.tile_pool(name="cm", bufs=2) as cmpool,
                    tc.tile_pool(name="sm", bufs=1) as smpool,
                    tc.tile_pool(name="rc", bufs=2) as rcpool,
                    tc.tile_pool(name="spsum", bufs=2, space="PSUM") as spsum,
                    tc.tile_pool(name="opsum", bufs=M, space="PSUM") as opsum,
                    tc.tile_pool(name="dpsum", bufs=1, space="PSUM") as dpsum,
                ):
                    ij_sb = smpool.tile([128, QBLK], F32, name="ij", tag="ij")
                    nc.sync.dma_start(out=ij_sb, in_=ij_d[:, :])
                    dl_sb = smpool.tile([128, nslots * maxcov], F32,
                                        name="dl", tag="dl")
                    nc.sync.dma_start(out=dl_sb, in_=dl_d[:, :])
                    ones_sb = smpool.tile([128, 2], BF16, name="ones",
                                          tag="ones")
                    nc.sync.dma_start(out=ones_sb, in_=ones_d[:, :])
                    touch(ones_sb)

                    def vt_load(i, h):
                        vt = vtpool.tile([128, 2, DHALF], BF16, name="vt",
                                         tag="vt")
                        nc.sync.dma_start(
                            out=vt,
                            in_=v_i[i * 128:i * 128 + 256,
                                    h * DHALF:(h + 1) * DHALF]
                            .rearrange("(t p) d -> p t d", p=128))
                        return vt

                    for s in range(nslots):
                        cov = COV[s]
                        qt = qT_sb[:, :, s * QBLK:(s + 1) * QBLK]
                        po = [opsum.tile([128, DHALF], F32, name="po",
                                         tag="po") for _ in range(M)]
                        pd = dpsum.tile([128, 2 * M], F32, name="pd",
                                        tag="pd")

                        def consume(i, at, vts, hh, dst, fresh_touch):
                            # attn@v + denominator matmuls for visit i
                            vt = vts[i // 2]
                            if fresh_touch:
                                touch(vt[:, 0, 0:2])
                            for m in range(M):
                                nc.tensor.matmul(
                                    dst[m],
                                    at[:, m * 128:(m + 1) * 128],
                                    vt[:, i % 2, :],
                                    start=(i == 0), stop=(i == cov - 1))
                            if hh == 0:
                                for m in range(M):
                                    nc.tensor.matmul(
                                        pd[:, 2 * m:2 * m + 2],
                                        at[:, m * 128:(m + 1) * 128],
                                        ones_sb[:, :],
                                        start=(i == 0 and m == 0),
                                        stop=(i == cov - 1 and m == M - 1))

                        # ---- sweep 1: scores + exp + attn@v(d-half 0) ----
                        a_tiles = []
                        vts = {0: vt_load(0, 0)}
                        prev = None
                        for i in range(cov):
                            if i % 2 == 0 and i + 2 < cov:
                                vts[(i + 2) // 2] = vt_load(i + 2, 0)
                            ps = spsum.tile([128, QBLK], F32, name="ps",
                                            tag="ps")
                            for ci in range(DC):
                                nc.tensor.matmul(
                                    ps,
                                    kT_sb[:, ci, i * 128:(i + 1) * 128],
                                    qt[:, ci, :],
                                    start=(ci == 0), stop=(ci == DC - 1))
                            at = apool.tile([128, QBLK], BF16, name="at",
                                            tag="at")
                            nc.scalar.activation(
                                out=at, in_=ps, func=AF.Exp, scale=scale)
                            if i >= cov - 2 * M:
                                cm = cmpool.tile([128, QBLK], BF16,
                                                 name="cm", tag="cm")
                                nc.vector.tensor_scalar(
                                    out=cm, in0=ij_sb,
                                    scalar1=dl_sb[:, s * maxcov + i:
                                                  s * maxcov + i + 1],
                                    scalar2=None, op0=ALU.is_le)
                                nc.vector.tensor_mul(out=at, in0=at, in1=cm)
                            a_tiles.append(at)
                            if prev is not None:
                                consume(prev, a_tiles[prev], vts, 0, po,
                                        prev % 2 == 0)
                            prev = i
                        consume(prev, a_tiles[prev], vts, 0, po,
                                prev % 2 == 0)

                        rc = rcpool.tile([128, 2 * M], F32, name="rc",
                                         tag="rc")
                        nc.vector.reciprocal(out=rc, in_=pd)

                        def norm_store(dst_po, h):
                            for m in range(M):
                                ot = otpool.tile([128, DHALF], F32,
                                                 name="ot", tag="ot")
                                nc.scalar.activation(
                                    out=ot, in_=dst_po[m], func=AF.Copy,
                                    scale=rc[:, 2 * m:2 * m + 1])
                                nc.scalar.dma_start(
                                    out=out_d[s * QBLK + m * 128:
                                              s * QBLK + (m + 1) * 128,
                                              h * DHALF:(h + 1) * DHALF],
                                    in_=ot)

                        norm_store(po, 0)

                        # ---- sweep 2: attn@v for remaining d-halves ----
                        for h in range(1, NH):
                            po2 = [opsum.tile([128, DHALF], F32, name="po",
                                              tag="po") for _ in range(M)]
                            vts = {0: vt_load(0, h)}
                            if cov > 2:
                                vts[1] = vt_load(2, h)
                            for i in range(cov):
                                if i % 2 == 0 and i + 4 < cov:
                                    vts[(i + 4) // 2] = vt_load(i + 4, h)
                                consume(i, a_tiles[i], vts, h, po2,
                                        i % 2 == 0)
                            norm_store(po2, h)
    # Bacc legalization: splits >1 sync waits into event semaphores.
    nc.compile()
    return nc


def host_core_inputs(x_b, Wq, Wk, Wv, q0s, S, D, QBLK, COV, half=None):
    """Input map for one core (half is accepted for test.py compat; unused)."""
    nslots = len(COV)
    maxcov = max(COV)
    xT = np.ascontiguousarray(x_b.T.astype(bfloat16))
    xTq = np.ascontiguousarray(
        np.concatenate([x_b[q0:q0 + QBLK] for q0 in q0s], axis=0).T
        .astype(bfloat16))
    ij = (np.arange(128, dtype=np.float32)[:, None]
          - np.arange(QBLK, dtype=np.float32)[None, :])
    ij = np.ascontiguousarray(np.broadcast_to(ij, (128, QBLK)))
    delta = np.empty((nslots, maxcov), dtype=np.float32)
    for s, q0 in enumerate(q0s):
        delta[s, :] = q0 - 128.0 * np.arange(maxcov, dtype=np.float32)
    delta = np.ascontiguousarray(
        np.broadcast_to(delta.reshape(1, -1), (128, nslots * maxcov)))
    return {
        "xT": xT, "xTq": xTq,
        "Wq": np.ascontiguousarray(Wq.astype(bfloat16)),
        "Wk": np.ascontiguousarray(Wk.astype(bfloat16)),
        "Wv": np.ascontiguousarray(Wv.astype(bfloat16)),
        "IJ": ij, "delta": delta,
        "ones": np.ones((128, 2), bfloat16),
    }


_NC_CACHE = {}


def _get_nc(key, cfg, **kw):
    if key not in _NC_CACHE:
        _NC_CACHE[key] = build_nc(**cfg, **kw)
    return _NC_CACHE[key]


def run_full(x, Wq, Wk, Wv, trace=False, trace_cores=None):
    """Run the 8-core kernel on full inputs; returns (out, BassKernelResults)."""
    cfg = CFG_FULL
    S, D, QBLK, COV = cfg["S"], cfg["D"], cfg["QBLK"], cfg["COV"]
    x = np.asarray(x, np.float32)
    Wq = np.asarray(Wq, np.float32)
    Wk = np.asarray(Wk, np.float32)
    Wv = np.asarray(Wv, np.float32)
    B = x.shape[0]
    assert (B, x.shape[1], x.shape[2]) == (B_FULL, S, D)

    nc = _get_nc("full", cfg)
    in_maps = []
    for b in range(B):
        for h in range(2):
            in_maps.append(host_core_inputs(
                x[b], Wq, Wk, Wv, Q0_FULL[h], S, D, QBLK, COV))
    res = run_bass_kernel_spmd(
        nc, in_maps, list(range(2 * B)), trace=trace,
        trace_cores=trace_cores)
    out = np.empty((B, S, D), np.float32)
    for b in range(B):
        for h in range(2):
            o = np.asarray(res.results[2 * b + h]["out"])
            for s, q0 in enumerate(Q0_FULL[h]):
                out[b, q0:q0 + QBLK] = o[s * QBLK:(s + 1) * QBLK]
    return out, res


def kernel(x, Wq, Wk, Wv):
    out, _ = run_full(x, Wq, Wk, Wv)
    return out
